# revision 29
# baseline (speedup 1.0000x reference)
"""ChebNet (K=3, 64->16->40) forward on 8 Trainium2 NeuronCores via Bass/Tile.

Algorithm (per L-hat pass, all feature-major / striped):
  L z = segment_sum(norm * z[row], col),  norm = -dis[row]*dis[col]
      = -dis[dst] * segsum( (dis*z)[row] )
Factorized passes (dis-scales folded into tables / epilogues):
  P1: table u2~=dis*(x@W12)       -> S1 ; v~   = u1~ - 2 dis^2 S1
  P2: table v~                    -> S2 ; out1 = dense - dis S2 + b1 ; h = relu
  P3: table h~=dis*h              -> S3 ; Tx1  = -dis S3 ; table4 = +dis^2 S3
  P4: table4                      -> S4'; Tx2  = 2 dis S4' - h
  out2 = h@W20 + Tx1@W21 + Tx2@W22 + b2 ; log_softmax rows

Sharding: nodes striped over 8 cores (12500 each, padded to S=8*B local,
B=1664). Edges owned by dest core, bucketed by (src band g, dest band k),
dest-sorted. Per (g,k) stream: [lead]+edges+pad. Device per chunk k:
  ap_gather msgs (per-group idx) -> in-place chunk-local prefix scan (DVE)
  -> ap_gather boundary samples at host-computed per-dest end positions
  -> PE combine over the 8 groups (ones matmul) -> adjacent-diff = per-dest
  segment sums. Cross-core table rebuild via DRAM AllGather each pass.

kernel(**inputs) takes FULL inputs, returns FULL [100000, 40] output.
"""
import os
import numpy as np

import concourse.bass as bass
import concourse.bacc as bacc
import concourse.mybir as mybir
import concourse.tile as tile
from concourse.alu_op_type import AluOpType
from concourse.bass_utils import run_bass_kernel_spmd

F32 = mybir.dt.float32
I16 = mybir.dt.int16
AF = mybir.ActivationFunctionType
NCORE = 8
HID = 16


class Cfg:
    def __init__(self, N=100000, IN=64, OUT=40, B=1664):
        self.N, self.IN, self.OUT, self.B = N, IN, OUT, B
        assert N % NCORE == 0
        self.SH = N // NCORE            # real nodes per core
        self.S = NCORE * B              # padded local nodes
        assert self.S >= self.SH and self.S % 256 == 0 and B % 128 == 0
        self.HS = self.S // 2           # xt half width
        self.EP = B + 1                 # boundary samples per chunk
        # 256-slot alignment: ap_gather idx slices must start 32B-aligned
        self.EP16 = ((self.EP + 255) // 256) * 256
        self.JC = B // 128              # 128-node chunks per band
        self.NCHIP = NCORE * self.JC    # final matmul chunks (104)


CFG = Cfg()


# ---------------------------------------------------------------- host side

def preprocess(cfg, edge_index):
    """Bucket/sort edges; build wrapped idx arrays and per-chunk sizes."""
    row = np.asarray(edge_index[0], dtype=np.int64)
    col = np.asarray(edge_index[1], dtype=np.int64)
    SH, B, S = cfg.SH, cfg.B, cfg.S

    c_n = lambda n: n // SH
    l_of = lambda n: n % SH
    g_of = lambda n: (n % SH) // B
    tbl_of = lambda n: (n // SH) * B + ((n % SH) % B)

    dc = c_n(col); dl = l_of(col); dk = g_of(col)
    sg = g_of(row); stbl = tbl_of(row)

    order = np.lexsort((dl, dk, sg, dc))
    dc, dl, dk, sg, stbl = dc[order], dl[order], dk[order], sg[order], stbl[order]

    cnt = np.zeros((NCORE, NCORE, NCORE), dtype=np.int64)
    np.add.at(cnt, (dc, sg, dk), 1)
    C = np.array([int(np.ceil((cnt[:, :, k].max() + 1) / 256) * 256)
                  for k in range(NCORE)])
    coff = np.concatenate([[0], np.cumsum(C)])
    GW = int(C.sum()) // 16
    EW = NCORE * cfg.EP16 // 16

    gidx = np.zeros((NCORE, 128, GW), dtype=np.int16)
    eidx = np.zeros((NCORE, 128, EW), dtype=np.int16)

    key = ((dc * NCORE + sg) * NCORE + dk)
    starts = np.searchsorted(key, np.arange(NCORE ** 3))
    stops = np.searchsorted(key, np.arange(NCORE ** 3) + 1)
    for c in range(NCORE):
        for g in range(NCORE):
            for k in range(NCORE):
                ki = (c * NCORE + g) * NCORE + k
                s0, e0 = int(starts[ki]), int(stops[ki])
                m = e0 - s0
                Ck = int(C[k])
                vals = np.zeros(Ck, dtype=np.int16)
                vals[1:1 + m] = stbl[s0:e0].astype(np.int16)
                # wrap: slot j -> [16g + j%16, coff//16 + j//16]
                w = vals.reshape(Ck // 16, 16).T
                gidx[c, 16 * g:16 * g + 16, coff[k] // 16: coff[k] // 16 + Ck // 16] = w

                dloc = dl[s0:e0] - k * B
                ends = np.cumsum(np.bincount(dloc, minlength=B))
                ev = np.zeros(cfg.EP16, dtype=np.int16)
                ev[1:1 + B] = ends.astype(np.int16)
                ev[1 + B:] = ev[B]
                we = ev.reshape(cfg.EP16 // 16, 16).T
                eidx[c, 16 * g:16 * g + 16,
                     k * cfg.EP16 // 16:(k + 1) * cfg.EP16 // 16] = we
    return C, coff, gidx, eidx


def host_arrays(cfg, x, edge_index, W1, b1, W2, b2):
    N, SH, B, S, HS = cfg.N, cfg.SH, cfg.B, cfg.S, cfg.HS
    x = np.asarray(x, dtype=np.float32)
    W1 = np.asarray(W1, dtype=np.float32); b1 = np.asarray(b1, dtype=np.float32)
    W2 = np.asarray(W2, dtype=np.float32); b2 = np.asarray(b2, dtype=np.float32)
    row = np.asarray(edge_index[0], dtype=np.int64)

    deg = np.bincount(row, minlength=N).astype(np.float32)
    dis = np.where(deg > 0, 1.0 / np.sqrt(np.maximum(deg, 1.0)), 0.0).astype(np.float32)

    C, coff, gidx, eidx = preprocess(cfg, edge_index)

    xs = x * dis[:, None]
    per_core = []
    p16 = np.arange(128) % 16
    # comb2 [128, 64]: par=0 cols 0..31: delta(f'==p%16); par=1: delta(f'==p%16+16)
    comb = np.zeros((128, 64), dtype=np.float32)
    comb[np.arange(128), p16] = 1.0
    comb[np.arange(128), 32 + p16 + 16] = 1.0
    # wmm_dev [128, 192]: rows i (mod 64) x blocks (t, par): [W|0] / [0|W] 32-wide
    Wt = [W1[1], W1[2], W1[0] - W1[2]]
    wmm = np.zeros((128, 192), dtype=np.float32)
    for t in range(3):
        for par in range(2):
            blk = np.zeros((cfg.IN, 32), dtype=np.float32)
            blk[:, 16 * par:16 * par + 16] = Wt[t]
            wmm[0:64, t * 64 + par * 32: t * 64 + (par + 1) * 32] = blk
            wmm[64:128, t * 64 + par * 32: t * 64 + (par + 1) * 32] = blk
    # w2_dev [128, 240]: rows 32q+i, blocks (par, t): rows sel by par
    w2 = np.zeros((128, 240), dtype=np.float32)
    for par in range(2):
        for t in range(3):
            blk = np.zeros((32, 40), dtype=np.float32)
            blk[16 * par:16 * par + 16, :] = W2[t]
            col = (par * 3 + t) * 40
            for q in range(4):
                w2[32 * q:32 * q + 32, col:col + 40] = blk
    b1c = b1[p16][:, None].astype(np.float32)                    # [128,1]
    b2r = np.tile(b2[None, :], (128, 1)).astype(np.float32)      # [128,40]

    for c in range(NCORE):
        xp = np.zeros((cfg.IN, S), dtype=np.float32)
        xsp = np.zeros((cfg.IN, S), dtype=np.float32)
        xp[:, :SH] = x[c * SH:(c + 1) * SH].T
        xsp[:, :SH] = xs[c * SH:(c + 1) * SH].T
        xt = np.concatenate([xp[:, :HS], xp[:, HS:]], axis=0)    # [128, HS]
        xts = np.concatenate([xsp[:, :HS], xsp[:, HS:]], axis=0)

        dl = np.zeros(S, dtype=np.float32)
        dl[:SH] = dis[c * SH:(c + 1) * SH]
        diss = dl.reshape(NCORE, B)[np.arange(128) // 16]        # [128,B]

        per_core.append(dict(
            xt=xt, xts=xts, diss=diss.astype(np.float32),
            gidx=gidx[c], eidx=eidx[c],
            wmm=wmm, w2=w2, comb=comb, b1c=b1c, b2r=b2r,
        ))
    return C, coff, per_core


# -------------------------------------------------------------- device side

def build_program(cfg, C, coff, reps=1):
    stage = int(os.environ.get("GNN_STAGE", "99"))
    N, IN, OUT, B, S, HS = cfg.N, cfg.IN, cfg.OUT, cfg.B, cfg.S, cfg.HS
    EP16 = cfg.EP16
    Cmax = int(C.max())
    GW = int(C.sum()) // 16
    EW = NCORE * EP16 // 16
    SUB = B // 4                      # diff subchunk width (416)

    nc = bacc.Bacc("TRN2", target_bir_lowering=False, debug=False,
                   enable_asserts=False, num_devices=NCORE)

    d_in = {}
    for nm, shape, dt in [
        ("xt", [128, HS], F32), ("xts", [128, HS], F32),
        ("diss", [128, B], F32),
        ("gidx", [128, GW], I16), ("eidx", [128, EW], I16),
        ("wmm", [128, 192], F32), ("w2", [128, 240], F32),
        ("comb", [128, 64], F32), ("b1c", [128, 1], F32),
        ("b2r", [128, OUT], F32),
    ]:
        d_in[nm] = nc.dram_tensor(nm, shape, dt, kind="ExternalInput").ap()
    out_d = nc.dram_tensor("out", [S, OUT], F32, kind="ExternalOutput").ap()

    with tile.TileContext(nc) as tc:
        with tc.tile_pool(name="main", bufs=1) as pool, \
             tc.tile_pool(name="msgp", bufs=2) as pool_msg, \
             tc.tile_pool(name="exp", bufs=1) as pool_ex, \
             tc.tile_pool(name="psum", bufs=2, space="PSUM") as pool_ps, \
             tc.tile_pool(name="dram", bufs=1, space="DRAM") as dram:

            # ---- static inputs into SBUF
            sb = {}
            for nm, shape, dt in [("diss", [128, B], F32),
                                  ("gidx", [128, GW], I16), ("eidx", [128, EW], I16),
                                  ("wmm", [128, 192], F32), ("w2", [128, 240], F32),
                                  ("comb", [128, 64], F32), ("b1c", [128, 1], F32),
                                  ("b2r", [128, OUT], F32)]:
                sb[nm] = pool.tile(shape, dt, name=f"sb_{nm}", tag=f"sb_{nm}")
                nc.sync.dma_start(sb[nm][:], d_in[nm][:])
            ones = pool.tile([128, 16], F32, tag="ones")
            nc.vector.memset(ones[:], 1.0)

            # persistent work tensors
            T = pool.tile([128, S], F32, tag="table")
            u1loc = pool.tile([128, B], F32, tag="u1loc")
            dloc = pool.tile([128, B], F32, tag="dloc")
            h = pool.tile([128, B], F32, tag="h")
            tx1 = pool.tile([128, B], F32, tag="tx1")
            sacc = pool.tile([128, B], F32, tag="sacc")
            exs = pool.tile([128, 4 * (B // 4 + 1)], F32, name="exs", tag="exs")
            agb = pool.tile([128, B], F32, tag="agb")
            BIGW = max(HS, cfg.NCHIP * OUT)
            mx = pool.tile([128, cfg.NCHIP], F32, tag="mx")
            sm = pool.tile([128, cfg.NCHIP], F32, tag="sm")

            def allgather(src):
                """src [128,B] SBUF -> T [128,S] SBUF via DRAM collective."""
                ag_in = dram.tile([128, B], F32, name="ag_in", tag="ag_in")
                ag_out = dram.tile([NCORE, 128, B], F32, name="ag_out",
                                   tag="ag_out", addr_space="Shared")
                nc.sync.dma_start(ag_in[:], src[:])
                if os.environ.get("GNN_NO_CC") == "1":
                    nc.sync.dma_start(ag_out[0, :, :], ag_in[:])
                else:
                    nc.gpsimd.collective_compute(
                        "AllGather", AluOpType.bypass,
                        replica_groups=[list(range(NCORE))],
                        ins=[ag_in[:].opt()], outs=[ag_out[:].opt()])
                # DRAM [c,p,j] -> SBUF [p, c*B+j]
                nc.sync.dma_start(
                    T[:].rearrange("p (c j) -> p c j", c=NCORE),
                    ag_out[:].rearrange("c p j -> p c j"))

            def mm_set(dst, t3, rhs_t):
                """dst[:, :] [128,B] = per-band (Wt.T @ rhs) fp32, band pairs."""
                for cc in range(4):
                    ps3 = pool_ps.tile([128, 4, 512], F32, name="pp", tag="pp")
                    ps = ps3[:, 0, :]
                    for q in range(4):
                        for par in range(2):
                            g = 2 * q + par
                            half = g // 4
                            c0 = (g % 4) * B + cc * SUB
                            nc.tensor.matmul(
                                ps[32 * q:32 * q + 32, 0:SUB],
                                sb["wmm"][64 * half:64 * half + 64,
                                          t3 * 64 + par * 32: t3 * 64 + par * 32 + 32],
                                rhs_t[64 * half:64 * half + 64, c0:c0 + SUB],
                                start=(par == 0), stop=(par == 1),
                                tile_position=(64 * half, 32 * q))
                    nc.vector.tensor_copy(dst[:, cc * SUB:(cc + 1) * SUB],
                                          ps[:, 0:SUB])

            def lhat_pass(dst):
                """dst [128,B] = per-dest unscaled segment sums from table T.

                One PSUM tile per band pair: a shared tile would let pair-q's
                PSUM->SBUF copy race later pairs' PE writes to the same banks.
                """
                exsv = exs[:].rearrange("p (s j) -> p s j", s=4)
                pse = None
                for k in range(NCORE):
                    Ck = int(C[k])
                    q, par = k // 2, k % 2
                    if par == 0:
                        pse = pool_ps.tile([128, 4, 512], F32, name="pp",
                                           tag="pp")
                    m = pool_msg.tile([128, Cmax], F32, name="msg", tag="msg")
                    nc.gpsimd.ap_gather(
                        m[:, 0:Ck], T[:],
                        sb["gidx"][:, coff[k] // 16: coff[k] // 16 + Ck // 16],
                        channels=128, num_elems=S, d=1, num_idxs=Ck)
                    nc.vector.tensor_tensor_scan(
                        m[:, 0:Ck], ones[:, 0:1].broadcast_to([128, Ck]),
                        m[:, 0:Ck], 0.0, AluOpType.mult, AluOpType.add)
                    ex = pool_ex.tile([128, EP16], F32, name="ex", tag="ex")
                    nc.gpsimd.ap_gather(
                        ex[:], m[:, 0:Ck],
                        sb["eidx"][:, k * EP16 // 16:(k + 1) * EP16 // 16],
                        channels=128, num_elems=Ck, d=1, num_idxs=EP16)
                    for s4 in range(4):
                        nc.tensor.matmul(
                            pse[32 * q:32 * q + 32, s4, 0:SUB + 1],
                            sb["comb"][:, 32 * par:32 * par + 32],
                            ex[:, SUB * s4: SUB * s4 + SUB + 1],
                            start=(par == 0), stop=(par == 1),
                            tile_position=(0, 32 * q))
                    if par == 1:
                        p0 = 32 * q
                        nc.vector.tensor_copy(exsv[p0:p0 + 32, :, :],
                                              pse[p0:p0 + 32, :, 0:SUB + 1])
                        nc.vector.tensor_tensor(
                            dst[p0:p0 + 32, :].rearrange(
                                "p (s j) -> p s j", s=4),
                            exsv[p0:p0 + 32, :, 1:SUB + 1],
                            exsv[p0:p0 + 32, :, 0:SUB],
                            AluOpType.subtract)

            def dump(src_ap, W):
                flat = out_d[:].rearrange("n o -> (n o)")
                nc.sync.dma_start(
                    flat[0:128 * W].rearrange("(p w) -> p w", p=128),
                    src_ap[:, 0:W])

            diss = sb["diss"]
            for _rep in range(reps):
                # ---- phase 0: u2~ table first (unblocks AG), then u1~, dense
                xt_t = pool.tile([128, BIGW], F32, name="xt_t", tag="bigbuf", bufs=2)
                xts_t = pool.tile([128, BIGW], F32, name="xts_t", tag="bigbuf", bufs=2)
                nc.sync.dma_start(xts_t[:, 0:HS], d_in["xts"][:])
                nc.sync.dma_start(xt_t[:, 0:HS], d_in["xt"][:])
                mm_set(agb, 1, xts_t)
                allgather(agb)
                mm_set(u1loc, 0, xts_t)
                mm_set(dloc, 2, xt_t)

                if stage < 1:
                    dump(T, min(S, (cfg.S * OUT) // 128))
                    continue
                if 10 <= stage <= 22:
                    # debug: partial lhat internals
                    KDBG = 3 if stage >= 20 else 2
                    KTGT = 2 if stage >= 20 else (1 if stage >= 15 else 0)
                    pse = pool_ps.tile([128, 4, 512], F32, name="pp", tag="pp")
                    dmp = None
                    for k in range(KDBG):
                        Ck = int(C[k])
                        m = pool_msg.tile([128, Cmax], F32, name="msg", tag="msg")
                        nc.gpsimd.ap_gather(
                            m[:, 0:Ck], T[:],
                            sb["gidx"][:, coff[k] // 16: coff[k] // 16 + Ck // 16],
                            channels=128, num_elems=S, d=1, num_idxs=Ck)
                        if stage in (10, 15, 20) and k == KTGT:
                            dmp = (m, Ck); break
                        nc.vector.tensor_tensor_scan(
                            m[:, 0:Ck], ones[:, 0:1].broadcast_to([128, Ck]),
                            m[:, 0:Ck], 0.0, AluOpType.mult, AluOpType.add)
                        if stage in (11, 16, 21) and k == KTGT:
                            dmp = (m, Ck); break
                        ex = pool_ex.tile([128, EP16], F32, name="ex", tag="ex")
                        nc.gpsimd.ap_gather(
                            ex[:], m[:, 0:Ck],
                            sb["eidx"][:, k * EP16 // 16:(k + 1) * EP16 // 16],
                            channels=128, num_elems=Ck, d=1, num_idxs=EP16)
                        if stage in (12, 17, 22) and k == KTGT:
                            dmp = (ex, EP16); break
                        q, par = k // 2, k % 2
                        for s4 in range(4):
                            nc.tensor.matmul(
                                pse[32 * q:32 * q + 32, s4, 0:SUB + 1],
                                sb["comb"][:, 32 * par:32 * par + 32],
                                ex[:, SUB * s4: SUB * s4 + SUB + 1],
                                start=(par == 0), stop=(par == 1),
                                tile_position=(0, 32 * q))
                    if stage >= 13 and dmp is None:
                        exsv = exs[:].rearrange("p (s j) -> p s j", s=4)
                        nc.vector.tensor_copy(exsv[0:32, :, :],
                                              pse[0:32, :, 0:SUB + 1])
                        nc.vector.tensor_tensor(
                            sacc[0:32, :].rearrange("p (s j) -> p s j", s=4),
                            exsv[0:32, :, 1:SUB + 1],
                            exsv[0:32, :, 0:SUB],
                            AluOpType.subtract)
                        dmp = (sacc, B)
                    dump(dmp[0], min(dmp[1], (cfg.S * OUT) // 128))
                    continue
                # ---- P1
                lhat_pass(sacc)
                if stage == 14:
                    dump(sacc, min(B, (cfg.S * OUT) // 128))
                    continue
                nc.vector.tensor_mul(sacc[:], sacc[:], diss[:])
                nc.vector.tensor_mul(sacc[:], sacc[:], diss[:])
                nc.vector.scalar_tensor_tensor(
                    agb[:], sacc[:], -2.0, u1loc[:],
                    AluOpType.mult, AluOpType.add)
                allgather(agb)

                if stage < 2:
                    dump(T, min(S, (cfg.S * OUT) // 128))
                    continue
                # ---- P2
                lhat_pass(sacc)
                nc.vector.tensor_mul(sacc[:], sacc[:], diss[:])
                nc.vector.tensor_sub(h[:], dloc[:], sacc[:])
                nc.vector.tensor_tensor(
                    h[:], h[:], sb["b1c"][:, 0:1].broadcast_to([128, B]),
                    AluOpType.add)
                nc.scalar.activation(h[:], h[:], AF.Relu)
                nc.vector.tensor_mul(agb[:], h[:], diss[:])
                allgather(agb)

                if stage < 3:
                    dump(T, min(S, (cfg.S * OUT) // 128))
                    continue
                # ---- P3
                lhat_pass(sacc)
                nc.vector.tensor_mul(sacc[:], sacc[:], diss[:])   # dis*S3
                nc.vector.tensor_scalar_mul(tx1[:], sacc[:], -1.0)
                nc.vector.tensor_mul(agb[:], sacc[:], diss[:])    # dis^2*S3
                allgather(agb)

                if stage < 4:
                    dump(T, min(S, (cfg.S * OUT) // 128))
                    continue
                # ---- P4 (tx2 kept in agb)
                lhat_pass(sacc)
                nc.vector.scalar_tensor_tensor(
                    sacc[:], sacc[:], 2.0, diss[:],
                    AluOpType.mult, AluOpType.mult)               # 2 dis S4'
                nc.vector.tensor_sub(agb[:], sacc[:], h[:])       # Tx2

                if stage < 5:
                    dump(agb, min(B, (cfg.S * OUT) // 128))
                    continue
                # ---- final matmuls: out2 chunks [128,40]
                o2 = pool.tile([128, BIGW], F32, name="o2", tag="bigbuf", bufs=2)
                e2 = pool.tile([128, BIGW], F32, name="e2", tag="bigbuf", bufs=2)
                nci = cfg.NCHIP
                assert nci % 4 == 0
                for ti in range(nci // 4):
                    pso = pool_ps.tile([128, 4, 512], F32, name="pp", tag="pp")
                    for w4 in range(4):
                        ci = ti * 4 + w4
                        g, jc = ci // cfg.JC, ci % cfg.JC
                        q, par = g // 2, g % 2
                        for t3, ten in enumerate([h, tx1, agb]):
                            nc.tensor.matmul(
                                pso[:, w4, 0:OUT],
                                ten[32 * q:32 * q + 32, jc * 128:(jc + 1) * 128],
                                sb["w2"][32 * q:32 * q + 32,
                                         (par * 3 + t3) * OUT:(par * 3 + t3 + 1) * OUT],
                                start=(t3 == 0), stop=(t3 == 2),
                                tile_position=(32 * q, 0))
                    nc.vector.tensor_copy(
                        o2[:, ti * 4 * OUT:(ti + 1) * 4 * OUT].rearrange(
                            "p (c o) -> p c o", o=OUT),
                        pso[:, :, 0:OUT])

                if stage < 6:
                    nc.sync.dma_start(out_d[0:128, :], o2[0:128, 0:OUT])
                    continue
                # ---- add b2 then log_softmax on [128, NCHIP, OUT]
                o2v = o2[:, 0:nci * OUT].rearrange("p (c o) -> p c o", o=OUT)
                nc.vector.tensor_tensor(
                    o2v, o2v,
                    sb["b2r"][:].unsqueeze(1).broadcast_to([128, nci, OUT]),
                    AluOpType.add)
                nc.vector.reduce_max(mx[:].unsqueeze(2), o2v,
                                     axis=mybir.AxisListType.X)
                nc.vector.tensor_tensor(
                    o2v, o2v,
                    mx[:].unsqueeze(2).broadcast_to([128, nci, OUT]),
                    AluOpType.subtract)
                nc.scalar.activation(e2[:, 0:nci * OUT], o2[:, 0:nci * OUT], AF.Exp)
                nc.vector.reduce_sum(sm[:].unsqueeze(2),
                                     e2[:, 0:nci * OUT].rearrange("p (c o) -> p c o", o=OUT),
                                     axis=mybir.AxisListType.X)
                nc.scalar.activation(sm[:], sm[:], AF.Ln)
                nc.vector.tensor_tensor(
                    e2[:, 0:nci * OUT].rearrange("p (c o) -> p c o", o=OUT), o2v,
                    sm[:].unsqueeze(2).broadcast_to([128, nci, OUT]),
                    AluOpType.subtract)
                if stage < 7:
                    nc.sync.dma_start(out_d[0:128, :], e2[0:128, 0:OUT])
                    continue
                # node l = g*B + jc*128 + p ; sbuf col = (g*JC+jc)*OUT + o
                nc.sync.dma_start(
                    out_d[:].rearrange("(g jc p) o -> p g jc o",
                                       g=NCORE, jc=cfg.JC, p=128),
                    e2[:, 0:nci * OUT].rearrange("p (g jc o) -> p g jc o",
                                                 g=NCORE, jc=cfg.JC))
    nc.compile()
    return nc


# --------------------------------------------------------------- execution

_CACHE = {}


def _get_program(cfg, C, coff, reps):
    key = (tuple(C.tolist()), reps, cfg.N, cfg.B)
    if key not in _CACHE:
        _CACHE[key] = build_program(cfg, C, coff, reps=reps)
    return _CACHE[key]


def run_on_hw(cfg, inputs, reps=1):
    C, coff, per_core = host_arrays(
        cfg, inputs["x"], inputs["edge_index"], inputs["W1"], inputs["b1"],
        inputs["W2"], inputs["b2"])
    nc = _get_program(cfg, C, coff, reps)
    in_maps = [dict(pc) for pc in per_core]
    res = run_bass_kernel_spmd(nc, in_maps, list(range(NCORE)))
    outs = []
    for c in range(NCORE):
        outs.append(res.results[c]["out"][:cfg.SH])
    return np.concatenate(outs, axis=0)


def kernel(x, edge_index, W1, b1, W2, b2):
    cfg = CFG
    out = run_on_hw(cfg, dict(x=x, edge_index=edge_index, W1=W1, b1=b1,
                              W2=W2, b2=b2), reps=1)
    return out.astype(np.float32)
